# revision 36
# baseline (speedup 1.0000x reference)
"""Trainium2 Bass kernel for nn_Loss_Function_90452011253875.

Detection-style loss: threshold matching (init proposals vs GT lines in
normalized (theta, radius) space), masked regression loss, softmax focal
loss (gamma=2).  Sharding: data-parallel over batch — each of 8 cores
processes 8 images and emits a partial [2] loss; the host sums partials.

Device algorithm (fp16 on-chip, threshold-scaled units):
  host precomputes  ti/TH, ri/TH, p0/TH, p1/TH  (fp16) and the
  F-replicated scaled GT rows (invalid GTs shifted +30000 so no match).
  per batch:  one quad tensor_tensor subtract [P,2,2,G,F] (2x DVE mode)
  -> Act abs -> max -> is_lt(.,1) = cond -> PE 24 identity matmuls
  accumulate match counts in PSUM -> is_gt gives gt flags; mask-mult
  (es * cond) -> Act Square(scale=TH) with accum_out gives the masked
  regression sums.  Focal loss on [P, 1024] from host-precomputed
  d = c1 - c0 via  -sigmoid(u)^2 * softplus(u),  u = (1-2*gt)*d.
"""
import os
import sys

for _p in ("/opt/trn_rl_repo", "/root/.axon_site/_ro/trn_rl_repo", "/root/.axon_site"):
    if os.path.isdir(_p) and _p not in sys.path:
        sys.path.append(_p)

import numpy as np

import concourse.bass as bass
import concourse.tile as tile
from concourse import bacc, mybir
from concourse.bass_utils import run_bass_kernel_spmd

F32 = mybir.dt.float32
F16 = mybir.dt.float16
Alu = mybir.AluOpType
Act = mybir.ActivationFunctionType

B, N, G = 64, 16384, 24
NCORES = 8
BPC = B // NCORES
P = 128
F = N // P
GF = G * F
NF = BPC * F

MAX_THETA = 90.0
MAX_RADIUS = 400.0
TH_T = 3.0 / MAX_THETA
TH_R = 20.0 / MAX_RADIUS
W_CLS = 2.0
W_REG = 5.0
PAD = -1000.0
SHIFT = 30000.0
RSC2 = W_REG / (2.0 * B)          # regression scale folded into accums
RSC = float(np.sqrt(RSC2))
FSC = W_CLS / (B * N)             # focal scale folded into the stt accum

_PROGRAM = None
_LAST_RESULTS = None


def _build_program():
    nc = bacc.Bacc("TRN2", target_bir_lowering=False, debug=False,
                   enable_asserts=False, num_devices=NCORES)

    ct_d = nc.dram_tensor("ct", [BPC, P, 4 * F], F16, kind="ExternalInput").ap()
    tr_d = nc.dram_tensor("tr", [BPC, P, 2 * GF], F16, kind="ExternalInput").ap()
    d_d = nc.dram_tensor("d", [P, NF], F16, kind="ExternalInput").ap()
    id_d = nc.dram_tensor("ident", [P, P], F16, kind="ExternalInput").ap()
    out_d = nc.dram_tensor("out", [P, 18], F32, kind="ExternalOutput").ap()

    from contextlib import ExitStack
    with tile.TileContext(nc) as tc, ExitStack() as ctx:
        inp = ctx.enter_context(tc.tile_pool(name="inp", bufs=3))
        trp = ctx.enter_context(tc.tile_pool(name="trp", bufs=3))
        dtqp = ctx.enter_context(tc.tile_pool(name="dtqp", bufs=2))
        atp = ctx.enter_context(tc.tile_pool(name="atp", bufs=2))
        condp = ctx.enter_context(tc.tile_pool(name="condp", bufs=2))
        mep = ctx.enter_context(tc.tile_pool(name="mep", bufs=2))
        persist = ctx.enter_context(tc.tile_pool(name="persist", bufs=1))
        small = ctx.enter_context(tc.tile_pool(name="small", bufs=1))
        psum = ctx.enter_context(tc.tile_pool(name="psum", bufs=4, space="PSUM"))

        ct_t = [None] * BPC
        tr_t = [None] * BPC
        dtq_t = [None] * BPC
        cps_t = [None] * BPC

        def load(b, split=False):
            tr_t[b] = trp.tile([P, 2 * GF], F16, name="tr_sb", tag="tr")
            if split:
                nc.sync.dma_start(tr_t[b][:, 0:GF], tr_d[b, :, 0:GF])
                ct_t[b] = inp.tile([P, 4 * F], F16, name="ct_sb", tag="ct")
                nc.sync.dma_start(ct_t[b][:], ct_d[b])
                nc.sync.dma_start(tr_t[b][:, GF:2 * GF], tr_d[b, :, GF:2 * GF])
            else:
                nc.sync.dma_start(tr_t[b][:], tr_d[b])
                ct_t[b] = inp.tile([P, 4 * F], F16, name="ct_sb", tag="ct")
                nc.sync.dma_start(ct_t[b][:], ct_d[b])

        load(0, split=True)
        load(1)
        ident = persist.tile([P, P], F16)
        nc.sync.dma_start(ident[:], id_d)
        d_all = persist.tile([P, NF], F16)
        nc.sync.dma_start(d_all[:], d_d)
        gt_all = persist.tile([P, NF], F16)
        racc = persist.tile([P, 18], F32)
        facc2 = racc[:, 16:18]

        at2_t = [None] * BPC

        def quad_sub(b, split=False):
            dtq_t[b] = dtqp.tile([P, 4 * GF], F16, name="dtq", tag="dtq")
            ct_bc = (ct_t[b][:].rearrange("p (h r f) -> p h r f", h=2, r=2)
                     .unsqueeze(3).broadcast_to([P, 2, 2, G, F]))
            tr_bc = (tr_t[b][:].rearrange("p (h g f) -> p h g f", h=2, g=24)
                     .unsqueeze(2).broadcast_to([P, 2, 2, G, F]))
            ov = dtq_t[b][:].rearrange("p (h r g f) -> p h r g f", h=2, r=2, g=G)
            dtqv = dtq_t[b][:].rearrange("p (h r g f) -> p h r g f", h=2, r=2, g=G)
            at2_t[b] = atp.tile([P, 2 * GF], F16, name="at2", tag="at2")
            at2v = at2_t[b][:].rearrange("p (h g f) -> p h g f", h=2, g=24)
            if split:
                # theta half can start as soon as the theta GT rows land
                nc.vector.tensor_tensor(ov[:, 0:1], ct_bc[:, 0:1], tr_bc[:, 0:1],
                                        Alu.subtract)
                nc.scalar.activation(at2v[:, 0:1], dtqv[:, 0:1, 0], Act.Abs)
                nc.vector.tensor_tensor(ov[:, 1:2], ct_bc[:, 1:2], tr_bc[:, 1:2],
                                        Alu.subtract)
                nc.scalar.activation(at2v[:, 1:2], dtqv[:, 1:2, 0], Act.Abs)
            else:
                nc.vector.tensor_tensor(ov, ct_bc, tr_bc, Alu.subtract)
                # abs here so the Act queue never blocks next batch's max
                nc.scalar.activation(at2v, dtqv[:, :, 0], Act.Abs)

        def tail(b):
            dtqv = dtq_t[b][:].rearrange("p (h r g f) -> p h r g f", h=2, r=2, g=G)
            at2v = at2_t[b][:].rearrange("p (h g f) -> p h g f", h=2, g=24)

            m = condp.tile([P, GF], F16, name="mx", tag="mx")
            nc.vector.tensor_tensor(m[:], at2v[:, 0].rearrange("p g f -> p (g f)"),
                                    at2v[:, 1].rearrange("p g f -> p (g f)"), Alu.max)
            cond = condp.tile([P, GF], F16, name="cond", tag="cond")
            nc.vector.tensor_scalar(cond[:], m[:], 1.0, None, Alu.is_lt)

            cv = cond[:].rearrange("p (g f) -> p g f", g=G)
            cps_t[b] = psum.tile([P, F], F32, name="cps", tag="cps")
            for g in range(G):
                nc.tensor.matmul(cps_t[b][:], lhsT=ident[:], rhs=cv[:, g],
                                 start=(g == 0), stop=(g == G - 1))

            me = mep.tile([P, 2 * GF], F16, name="me", tag="me")
            mev = me[:].rearrange("p (h g f) -> p h g f", h=2, g=24)
            # split theta/rho so the Act square can start on theta early;
            # fold TH^2 * W_REG/(2B) into the squares so no final rescale
            nc.vector.tensor_tensor(mev[:, 0], dtqv[:, 0, 1], cv, Alu.mult)
            if b != BPC - 1:
                nc.scalar.activation(me[:, 0:GF], mev[:, 0], Act.Square,
                                     scale=TH_T * RSC,
                                     accum_out=racc[:, 2 * b:2 * b + 1])
            nc.vector.tensor_tensor(mev[:, 1], dtqv[:, 1, 1], cv, Alu.mult)
            if b == BPC - 1:
                # last batch: rho square deferred to the epilogue (on DVE),
                # emitted after the focal head so sigmoid/ln overlap it
                melast[0] = mev
            else:
                nc.scalar.activation(me[:, GF:2 * GF], mev[:, 1], Act.Square,
                                     scale=TH_R * RSC,
                                     accum_out=racc[:, 2 * b + 1:2 * b + 2])

        def isgt(b):
            # deferred two iterations so the PE has a long window to drain
            # the accumulating count matmuls before DVE reads the PSUM
            nc.vector.tensor_scalar(gt_all[:, b * F:(b + 1) * F], cps_t[b][:],
                                    0.0, None, Alu.is_gt)

        SPLITC = 7 * F
        uref = [None, None]

        melast = [None]

        def focal_half(h, part="all"):
            # picked = sigmoid(u)^2 * softplus(u);  softplus(u) = -ln(sigmoid(-u))
            s = slice(0, SPLITC) if h == 0 else slice(SPLITC, NF)
            HF = s.stop - s.start
            if part in ("all", "head"):
                sgn = small.tile([P, HF], F16, name="sgn", tag="sgn%d" % h)
                nc.vector.tensor_scalar(sgn[:], gt_all[:, s], -2.0, 1.0,
                                        Alu.mult, Alu.add)
                u = small.tile([P, HF], F16, name="u", tag="u%d" % h)
                nc.vector.tensor_tensor(u[:], d_all[:, s], sgn[:], Alu.mult)
                uref[h] = u
                if part == "head":
                    return
            u = uref[h]
            sgneg = small.tile([P, HF], F16, name="sgneg", tag="sgneg%d" % h)
            nc.scalar.activation(sgneg[:], u[:], Act.Sigmoid, scale=-1.0)
            lnneg = small.tile([P, HF], F16, name="lnneg", tag="lnneg%d" % h)
            nc.scalar.activation(lnneg[:], sgneg[:], Act.Ln)
            om = small.tile([P, HF], F16, name="om", tag="om%d" % h)
            nc.vector.tensor_scalar(om[:], sgneg[:], -1.0, 1.0, Alu.mult, Alu.add)
            pr2 = small.tile([P, HF], F16, name="pr2", tag="pr2%d" % h)
            nc.vector.tensor_tensor(pr2[:], om[:], om[:], Alu.mult)
            waste = small.tile([P, HF], F16, name="waste", tag="waste%d" % h)
            nc.vector.scalar_tensor_tensor(waste[:], lnneg[:], -FSC, pr2[:],
                                           Alu.mult, Alu.mult,
                                           accum_out=facc2[:, h:h + 1])
            del waste

        # software pipeline: quad_sub(b+1) (DVE sub + Act abs) is emitted
        # before tail(b); isgt(b) lands two iterations later so the PE has a
        # wide window to drain the accumulating count matmuls.
        quad_sub(0, split=True)
        for b in range(BPC):
            if b + 2 < BPC:
                load(b + 2)
            if b + 1 < BPC:
                quad_sub(b + 1, split=True)
            if b >= 3:
                isgt(b - 3)
            if b == BPC - 1:
                isgt(b - 2)
                isgt(b - 1)
                focal_half(0)
            tail(b)
        isgt(BPC - 1)
        # last batch's theta square on Act, emitted here so it runs while
        # DVE does the rho square and the focal tail
        b7 = BPC - 1
        mel = melast[0]
        nc.scalar.activation(mel[:, 0].rearrange("p g f -> p (g f)"),
                             mel[:, 0], Act.Square, scale=TH_T * RSC,
                             accum_out=racc[:, 2 * b7:2 * b7 + 1])
        focal_half(1, part="head")
        nc.vector.scalar_tensor_tensor(
            melast[0][:, 1].rearrange("p g f -> p (g f)"),
            melast[0][:, 1].rearrange("p g f -> p (g f)"),
            TH_R * TH_R * RSC2,
            melast[0][:, 1].rearrange("p g f -> p (g f)"),
            Alu.mult, Alu.mult,
            accum_out=racc[:, 2 * b7 + 1:2 * b7 + 2])
        focal_half(1, part="rest")

        # ---- ship raw per-partition partials; host does the final sums
        # (scales already folded into the accumulators)
        nc.sync.dma_start(out_d, racc[:])

    nc.compile()
    return nc


def _get_program():
    global _PROGRAM
    if _PROGRAM is None:
        _PROGRAM = _build_program()
    return _PROGRAM


def _host_prep(cls, params, params_init, tgt_params, pts):
    """Per-core input prep: scale to threshold units, fp16, layouts."""
    cls = np.asarray(cls, dtype=np.float32)
    params = np.asarray(params, dtype=np.float32)
    params_init = np.asarray(params_init, dtype=np.float32)
    tgt_params = np.asarray(tgt_params, dtype=np.float32)
    pts = np.asarray(pts, dtype=np.float32)

    ident = np.eye(P, dtype=np.float16)
    in_maps = []
    for c in range(NCORES):
        s = slice(c * BPC, (c + 1) * BPC)
        pi = params_init[s]                       # [BPC, N, 2]
        pp = params[s]
        tg = tgt_params[s]                        # [BPC, G, 2]
        pt = pts[s]
        cl = cls[s]

        # center scaled coords (theta: [0,30]-15, rho: [0,20]-10) to halve
        # the fp16 rounding band around the +-1 match threshold
        ti = (pi[..., 0] / TH_T - 15.0).reshape(BPC, P, F)
        ri = (pi[..., 1] / TH_R - 10.0).reshape(BPC, P, F)
        p0 = (pp[..., 0] / TH_T - 15.0).reshape(BPC, P, F)
        p1 = (pp[..., 1] / TH_R - 10.0).reshape(BPC, P, F)
        ct = np.stack([ti, p0, ri, p1], axis=2)   # [BPC, P, 4, F]
        ct = np.ascontiguousarray(ct.reshape(BPC, P, 4 * F), dtype=np.float16)

        valid = pt[..., 0] != PAD                 # [BPC, G]
        t_s = ((tg[..., 0] + MAX_THETA) / (2 * MAX_THETA)) / TH_T - 15.0
        r_s = ((tg[..., 1] + MAX_RADIUS) / (2 * MAX_RADIUS)) / TH_R - 10.0
        t_s = np.where(valid, t_s, SHIFT)
        r_s = np.where(valid, r_s, SHIFT)
        tr = np.stack([t_s, r_s], axis=1)         # [BPC, 2, G]
        tr16 = np.broadcast_to(tr.astype(np.float16)[:, :, :, None],
                               (BPC, 2, G, F)).reshape(BPC, 1, 2 * GF)
        tr16 = np.ascontiguousarray(
            np.broadcast_to(tr16, (BPC, P, 2 * GF)))

        d = (cl[..., 1] - cl[..., 0]).reshape(BPC, P, F)
        d = np.ascontiguousarray(
            d.transpose(1, 0, 2).reshape(P, NF), dtype=np.float16)

        in_maps.append({"ct": ct, "tr": tr16, "d": d, "ident": ident})
    return in_maps


def kernel(cls, params, params_init, tgt_params, pts, profile=False):
    global _LAST_RESULTS
    nc = _get_program()
    in_maps = _host_prep(cls, params, params_init, tgt_params, pts)
    res = run_bass_kernel_spmd(nc, in_maps, list(range(NCORES)), trace=False)
    _LAST_RESULTS = res
    total = np.zeros(2, dtype=np.float64)
    for c in range(NCORES):
        acc = res.results[c]["out"].astype(np.float64)   # [P, 18]
        total[0] += acc[:, 16:18].sum()
        total[1] += acc[:, 0:16].sum()
    return total.astype(np.float32)
